# revision 32
# baseline (speedup 1.0000x reference)
"""Trainium2 Bass kernel for nn_CrossAttention.

Sharding: data-parallel over batch (B=8 -> 8 cores, one batch element per
core). No collectives. Host pre-transposes activations/weights into
contraction-on-partition layouts and casts to bf16.

Performance structure:

  * context compaction: masked-out kv positions (typically ~50%) are
    gathered away on the host; the kernel is compiled at runtime for the
    padded valid length (multiple of 128), cutting scores/PV/K/V matmul
    rows and softmax-exp work roughly in half with bit-identical math for
    the surviving positions (padding rows are killed by the exp bias);
  * attention inner loop is software-pipelined: scores(h,t+1) issue ahead
    of PV(h,t), with exp(h,t) on ACT in between (2-deep PSUM rotation);
  * projection matmul groups (Q of the next chunk, out-proj of the
    previous chunk, normalize broadcasts) are interleaved as PE filler so
    the tensor engine never idles;
  * no DRAM bounces: all partition broadcasts are PE selector matmuls;
  * the k-side RMS factor rides the exp's per-partition scale operand, so
    khat needs no normalize multiply at all;
  * softmax denominators are staged out and inverted per chunk on DVE;
    the projection bias is a DVE add (no PE bias matmuls).
"""

import sys

for _p in ("/opt/trn_rl_repo",):
    if _p not in sys.path:
        sys.path.insert(0, _p)

import numpy as np
import ml_dtypes

import concourse.bass as bass
import concourse.mybir as mybir
import concourse.tile as tile
from concourse import bacc
from concourse import bass_utils

BF16 = mybir.dt.bfloat16
F32 = mybir.dt.float32
BFNP = ml_dtypes.bfloat16

B, LQ, LKV, D, H = 8, 2048, 1024, 1024, 16
HD = D // H          # 64
P = 128              # partitions
DT = D // P          # 8 d-tiles
CH = 1024            # lq chunk
NCH = LQ // CH       # 2
EPS = 1e-6
NEG = -1.0e30

_CACHE = {}
LAST_RESULTS = None


def _patch_act_tables():
    """Restrict usable ACT function sets to natural_log_exp_and_others (it
    contains both Exp and Ln) so the table-load pass never alternates between
    exp_and_others / natural_log — each switch costs ~2.7us on ScalarE."""
    import concourse.hw_specs as hw_specs
    import concourse.bass_interp as bass_interp

    if getattr(_patch_act_tables, "_done", False):
        return
    orig = hw_specs.get_activation_tables

    def patched(module_arch):
        t = orig(module_arch)
        keep = "natural_log_exp_and_others"
        if keep in t:
            t = {k: (v if k == keep else set()) for k, v in t.items()}
        return t

    hw_specs.get_activation_tables = patched
    bacc.get_activation_tables = patched
    bass_interp.get_activation_tables = patched
    _patch_act_tables._done = True


def _build(lkv_e):
    _patch_act_tables()
    nc = bacc.Bacc("TRN2", target_bir_lowering=False, debug=False)

    kt = lkv_e // P
    ctxT_d = nc.dram_tensor("ctxT", (D, lkv_e), BF16, kind="ExternalInput").ap()
    xT_d = nc.dram_tensor("xT", (D, LQ), BF16, kind="ExternalInput").ap()
    wqT_d = nc.dram_tensor("wqT", (D, D), BF16, kind="ExternalInput").ap()
    wkT_d = nc.dram_tensor("wkT", (D, D), BF16, kind="ExternalInput").ap()
    wvT_d = nc.dram_tensor("wvT", (D, D), BF16, kind="ExternalInput").ap()
    wpT_d = nc.dram_tensor("wpT", (D, D), BF16, kind="ExternalInput").ap()
    bp_d = nc.dram_tensor("bp", (1, D), BF16, kind="ExternalInput").ap()
    mask_d = nc.dram_tensor("mask", (P, kt), F32, kind="ExternalInput").ap()
    selwq_d = nc.dram_tensor("selwq", (H, D), BF16, kind="ExternalInput").ap()
    selr_d = nc.dram_tensor("selr", (H, D), BF16, kind="ExternalInput").ap()
    out_d = nc.dram_tensor("out", (LQ, D), F32, kind="ExternalOutput").ap()

    with tile.TileContext(nc) as tc:
        _kernel_body(
            nc, tc, lkv_e, xT_d, ctxT_d, wqT_d, wkT_d, wvT_d, wpT_d, bp_d,
            mask_d, selwq_d, selr_d, out_d,
        )
    nc.compile()
    return nc


def _kernel_body(
    nc, tc, lkv_e, xT_d, ctxT_d, wqT_d, wkT_d, wvT_d, wpT_d, bp_d, mask_d,
    selwq_d, selr_d, out_d,
):
    import contextlib

    kt = lkv_e // P
    # K-projection free-dim slices of the compacted kv length
    kslices = []
    o = 0
    while o < lkv_e:
        w = min(512, lkv_e - o)
        kslices.append((o, w))
        o += w

    ctx = contextlib.ExitStack()
    with ctx:
        const = ctx.enter_context(tc.tile_pool(name="const", bufs=1))
        wpool = ctx.enter_context(tc.tile_pool(name="wpool", bufs=1))
        io = ctx.enter_context(tc.tile_pool(name="io", bufs=2))
        kv = ctx.enter_context(tc.tile_pool(name="kv", bufs=1))
        sqp = ctx.enter_context(tc.tile_pool(name="sqp", bufs=1))
        qp = ctx.enter_context(tc.tile_pool(name="qp", bufs=2))
        va = ctx.enter_context(tc.tile_pool(name="va", bufs=2))
        probs_pool = ctx.enter_context(tc.tile_pool(name="probs", bufs=3))
        small = ctx.enter_context(tc.tile_pool(name="small", bufs=1))
        den_pool = ctx.enter_context(tc.tile_pool(name="den", bufs=1))
        out_pool = ctx.enter_context(tc.tile_pool(name="outp", bufs=2))
        sc_ps = ctx.enter_context(tc.tile_pool(name="sc_ps", bufs=2, space="PSUM"))
        pv_ps = ctx.enter_context(tc.tile_pool(name="pv_ps", bufs=2, space="PSUM"))
        aux_ps = ctx.enter_context(tc.tile_pool(name="aux_ps", bufs=2, space="PSUM"))

        # ---- loads needed first: wk + compacted ctx (K-proj inputs) ----
        wk_sb = []
        for k in range(DT):
            t = wpool.tile([P, D], BF16, name=f"wk{k}", tag=f"wkp{k}")
            nc.sync.dma_start(t[:], wkT_d[P * k : P * (k + 1), :])
            wk_sb.append(t)
        ctx_sb = []
        for k in range(DT):
            t = io.tile([P, lkv_e], BF16, name=f"ctx{k}", tag=f"io{k}")
            nc.sync.dma_start(t[:], ctxT_d[P * k : P * (k + 1), :])
            ctx_sb.append(t)

        # ---- small constants ----
        mask_sb = const.tile([P, kt], F32, name="mask_sb")
        nc.sync.dma_start(mask_sb[:], mask_d[:])
        selwq_sb = const.tile([H, D], BF16, name="selwq_sb")
        nc.sync.dma_start(selwq_sb[:], selwq_d[:])
        selr = const.tile([H, D], BF16, name="selr")
        nc.sync.dma_start(selr[:], selr_d[:])
        bias_sb = const.tile([P, D], BF16, name="bias_sb")
        nc.sync.dma_start(bias_sb[:], bp_d[0:1, :].broadcast_to((P, D)))
        eps128 = const.tile([P, 1], F32, name="eps128")
        nc.vector.memset(eps128[:], EPS)
        zero128 = const.tile([P, 1], F32, name="zero128")
        nc.vector.memset(zero128[:], 0.0)
        eps16 = const.tile([H, 1], F32, name="eps16")
        nc.vector.memset(eps16[:], EPS)
        zero16 = const.tile([H, 1], F32, name="zero16")
        nc.vector.memset(zero16[:], 0.0)
        # sel16[m]: [128, 16] with col 2m set on partitions 0-63, col 2m+1 on
        # 64-127.  Used as rhs for k-stats and lhsT for q-stats.
        sel16 = []
        for m in range(DT):
            s = const.tile([P, H], BF16, name=f"sel{m}")
            nc.vector.memset(s[:], 0.0)
            nc.vector.memset(s[0:64, 2 * m : 2 * m + 1], 1.0)
            nc.vector.memset(s[64:128, 2 * m + 1 : 2 * m + 2], 1.0)
            sel16.append(s)

        # remaining weights: wv shares slots with the attn accumulators
        wv_sb = []
        for k in range(DT):
            t = va.tile([P, D], BF16, name=f"wv{k}", tag=f"va{k}")
            nc.sync.dma_start(t[:], wvT_d[P * k : P * (k + 1), :])
            wv_sb.append(t)
        wq_sb = []
        for k in range(DT):
            t = wpool.tile([P, D], BF16, name=f"wq{k}", tag=f"wq{k}")
            nc.sync.dma_start(t[:], wqT_d[P * k : P * (k + 1), :])
            wq_sb.append(t)

        khat = [kv.tile([P, lkv_e], BF16, name=f"khat{m}") for m in range(DT)]
        vsb = [kv.tile([P, H * (HD + 1)], BF16, name=f"vsb{m}") for m in range(kt)]

        def make_ps_alloc(wide):
            """PSUM allocator for projection groups.  While attention is NOT
            running (pre-attention stages, final tail), cycle through the idle
            sc/pv slots too, for a 6-deep rotation so the PE runs well ahead
            of the DVE copies.  During attention only the aux slots are safe."""
            import itertools

            if not wide:
                seq = itertools.cycle([(aux_ps, "mm")])
            else:
                seq = itertools.cycle(
                    [(aux_ps, "mm"), (sc_ps, "sc"), (aux_ps, "mm"), (pv_ps, "pv")]
                )

            def alloc(name):
                pool, tag = next(seq)
                return pool.tile([P, 512], F32, name=name, tag=tag)

            return alloc

        # ---------------- K projection ----------------
        pre_alloc = make_ps_alloc(True)
        sq_k = []
        for m in range(DT):
            for o, w in kslices:
                ps = pre_alloc("ps_k")
                for k in range(DT):
                    nc.tensor.matmul(
                        ps[:, 0:w],
                        wk_sb[k][:, P * m : P * (m + 1)],
                        ctx_sb[k][:, o : o + w],
                        start=(k == 0), stop=(k == DT - 1),
                    )
                nc.vector.tensor_copy(khat[m][:, o : o + w], ps[:, 0:w])
            sq = sqp.tile([P, lkv_e], BF16, name=f"sq{m}", tag=f"sq{m}")
            nc.vector.tensor_tensor(
                sq[:], khat[m][:], khat[m][:], mybir.AluOpType.mult
            )
            sq_k.append(sq)

        # wp loads into wk's slots once K-proj has consumed wk.
        wp_sb = []
        for k in range(DT):
            t = wpool.tile([P, D], BF16, name=f"wp{k}", tag=f"wkp{k}")
            nc.sync.dma_start(t[:], wpT_d[P * k : P * (k + 1), :])
            wp_sb.append(t)

        # ---------------- V projection ----------------
        for m in range(kt):
            for n in range(2):
                ps = pre_alloc("ps_v")
                for k in range(DT):
                    nc.tensor.matmul(
                        ps[:],
                        ctx_sb[k][:, P * m : P * (m + 1)],
                        wv_sb[k][:, 512 * n : 512 * (n + 1)],
                        start=(k == 0), stop=(k == DT - 1),
                    )
                v3 = vsb[m][:].rearrange("p (h e) -> p h e", e=HD + 1)
                nc.vector.tensor_copy(
                    v3[:, 8 * n : 8 * (n + 1), 0:HD],
                    ps[:].rearrange("p (h e) -> p h e", e=HD),
                )
            v3 = vsb[m][:].rearrange("p (h e) -> p h e", e=HD + 1)
            nc.vector.memset(v3[:, :, HD : HD + 1], 1.0)

        # ---------------- k-side RMS factors (transposed stats) ----------
        # rkps[kpos, 16t + h] = sum_d k[d, kpos]^2 for head h, lkv tile t.
        rkps = pv_ps.tile([P, H * kt], F32, name="rkps", tag="pv")
        for t in range(kt):
            for m in range(DT):
                nc.tensor.matmul(
                    rkps[:, H * t : H * (t + 1)],
                    sq_k[m][:, P * t : P * (t + 1)],
                    sel16[m][:],
                    start=(m == 0), stop=(m == DT - 1),
                )
        rkln = small.tile([P, H * kt], F32, name="rkln")
        nc.scalar.activation(
            rkln[:], rkps[:], mybir.ActivationFunctionType.Ln,
            bias=eps128[:], scale=1.0 / HD,
        )
        rk_sb = small.tile([P, H * kt], F32, name="rk_sb")
        nc.scalar.activation(
            rk_sb[:], rkln[:], mybir.ActivationFunctionType.Exp,
            bias=zero128[:], scale=-0.5,
        )

        # ---------------- per-chunk helpers ----------------
        qhat = {}      # chunk -> list of 8 [128, CH] bf16 tiles (normalized q)
        attn = {}      # chunk -> list of 8 [128, CH] bf16 tiles
        denall = {}    # chunk -> [16, CH] f32 denominators

        def emit_x_load(c):
            tiles = []
            for k in range(DT):
                t = io.tile([P, CH], BF16, name=f"x{c}_{k}", tag=f"io{k}")
                nc.sync.dma_start(
                    t[:], xT_d[P * k : P * (k + 1), CH * c : CH * (c + 1)]
                )
                tiles.append(t)
            return tiles

        def q_build_thunks(c, x_tiles, wide=False):
            """Thunk list: Q projection + RMS stats + normalize for chunk c."""
            ps_alloc = make_ps_alloc(wide)
            q_tiles = [
                qp.tile([P, CH], BF16, name=f"q{c}_{m}", tag=f"q{m}")
                for m in range(DT)
            ]
            qhat[c] = q_tiles
            sq_q = []
            thunks = []

            def proj_group(m, n):
                def f():
                    ps = ps_alloc("ps_q")
                    for k in range(DT):
                        nc.tensor.matmul(
                            ps[:],
                            wq_sb[k][:, P * m : P * (m + 1)],
                            x_tiles[k][:, 512 * n : 512 * (n + 1)],
                            start=(k == 0), stop=(k == DT - 1),
                        )
                    nc.vector.tensor_copy(
                        q_tiles[m][:, 512 * n : 512 * (n + 1)], ps[:]
                    )
                    if n == 1:
                        sq = sqp.tile([P, CH], BF16, name=f"sqq{c}_{m}", tag=f"sq{m}")
                        nc.vector.tensor_tensor(
                            sq[:], q_tiles[m][:], q_tiles[m][:],
                            mybir.AluOpType.mult,
                        )
                        sq_q.append(sq)
                return f

            for m in range(DT):
                for n in range(2):
                    thunks.append(proj_group(m, n))

            ln_sb = small.tile([H, CH], F32, name=f"lnq{c}", tag="lnq")
            rs_q = small.tile([H, CH], BF16, name=f"rsq{c}", tag="rsq")

            def stats():
                for n in range(2):
                    qs = ps_alloc("qs")[0:H, :]
                    for m in range(DT):
                        nc.tensor.matmul(
                            qs[:],
                            sel16[m][:],
                            sq_q[m][:, 512 * n : 512 * (n + 1)],
                            start=(m == 0), stop=(m == DT - 1),
                        )
                    nc.scalar.activation(
                        ln_sb[:, 512 * n : 512 * (n + 1)], qs[:],
                        mybir.ActivationFunctionType.Ln,
                        bias=eps16[:], scale=1.0 / HD,
                    )
                nc.scalar.activation(
                    rs_q[:], ln_sb[:], mybir.ActivationFunctionType.Exp,
                    bias=zero16[:], scale=-0.5,
                )

            thunks.append(stats)

            def bcast_group(m):
                def f():
                    for n in range(2):
                        rb = ps_alloc("rbq")
                        nc.tensor.matmul(
                            rb[:],
                            selwq_sb[:, P * m : P * (m + 1)],
                            rs_q[:, 512 * n : 512 * (n + 1)],
                            start=True, stop=True,
                        )
                        nc.vector.tensor_tensor(
                            q_tiles[m][:, 512 * n : 512 * (n + 1)],
                            q_tiles[m][:, 512 * n : 512 * (n + 1)],
                            rb[:],
                            mybir.AluOpType.mult,
                        )
                return f

            for m in range(DT):
                thunks.append(bcast_group(m))
            return thunks

        def normalize_thunks(c, tail=False):
            """Reciprocal of denominators + broadcast + apply to attn[c]."""
            thunks = []
            ps_alloc = make_ps_alloc(tail)
            recf = small.tile([H, CH], F32, name=f"recf{c}", tag="recf")
            rec16 = small.tile([H, CH], BF16, name=f"rec{c}", tag="rec16")

            def recip():
                nc.vector.reciprocal_approx_fast(recf[:], denall[c][:])
                nc.vector.tensor_copy(rec16[:], recf[:])

            thunks.append(recip)

            def rbr_group(m):
                def f():
                    for n in range(2):
                        rb = ps_alloc("rbr")
                        nc.tensor.matmul(
                            rb[:],
                            selr[:, P * m : P * (m + 1)],
                            rec16[:, 512 * n : 512 * (n + 1)],
                            start=True, stop=True,
                        )
                        nc.vector.tensor_tensor(
                            attn[c][m][:, 512 * n : 512 * (n + 1)],
                            attn[c][m][:, 512 * n : 512 * (n + 1)],
                            rb[:],
                            mybir.AluOpType.mult,
                        )
                return f

            for m in range(DT):
                thunks.append(rbr_group(m))
            return thunks

        def oproj_thunks(c, tail=False):
            thunks = []
            ps_alloc = make_ps_alloc(tail)

            def group(mq, n):
                def f():
                    ps = ps_alloc("ps_o")
                    for k in range(DT):
                        nc.tensor.matmul(
                            ps[:],
                            attn[c][k][:, P * mq : P * (mq + 1)],
                            wp_sb[k][:, 512 * n : 512 * (n + 1)],
                            start=(k == 0), stop=(k == DT - 1),
                        )
                    o_sb = out_pool.tile([P, 512], F32, name="o_sb", tag="o")
                    nc.vector.tensor_tensor(
                        o_sb[:], ps[:], bias_sb[:, 512 * n : 512 * (n + 1)],
                        mybir.AluOpType.add,
                    )
                    nc.sync.dma_start(
                        out_d[CH * c + P * mq : CH * c + P * (mq + 1),
                              512 * n : 512 * (n + 1)],
                        o_sb[:],
                    )
                return f

            for mq in range(CH // P):
                for n in range(2):
                    thunks.append(group(mq, n))
            return thunks

        def attention(c, fillers, last=False):
            """Attention for chunk c; pops filler thunks to keep PE fed."""
            att_tiles = [
                va.tile([P, CH], BF16, name=f"at{c}_{m}", tag=f"va{m}")
                for m in range(DT)
            ]
            attn[c] = att_tiles
            den = den_pool.tile([H, CH], F32, name=f"den{c}", tag="den")
            denall[c] = den

            # hold a few thunks back for after the last head, so the PE has
            # work while the final denominator -> reciprocal chain drains.
            fill_budget = len(fillers) / 20.0 / 2.0
            popped = 0.0
            for h in range(H):
                mt, off = h // 2, 64 * (h % 2)
                pv0 = pv_ps.tile([HD + 1, 512], F32, name="pv0", tag="pv")
                pv1 = pv_ps.tile([HD + 1, 512], F32, name="pv1", tag="pv")
                prs = []
                for t in range(kt):
                    if t == kt // 2:
                        popped += fill_budget
                        while fillers and popped >= 1.0:
                            fillers.pop(0)()
                            popped -= 1.0
                    sc = sc_ps.tile([P, CH], F32, name="sc", tag="sc")
                    for n in range(2):
                        nc.tensor.matmul(
                            sc[:, 512 * n : 512 * (n + 1)],
                            khat[mt][off : off + HD, P * t : P * (t + 1)],
                            qhat[c][mt][off : off + HD, 512 * n : 512 * (n + 1)],
                            start=True, stop=True,
                        )
                    pr = probs_pool.tile([P, CH], BF16, name="pr", tag="pr")
                    nc.scalar.activation(
                        pr[:], sc[:], mybir.ActivationFunctionType.Exp,
                        bias=mask_sb[:, t : t + 1],
                        scale=rk_sb[:, H * t + h : H * t + h + 1],
                    )
                    prs.append(pr)
                    if t >= 1:
                        for half, pvt in ((0, pv0), (1, pv1)):
                            nc.tensor.matmul(
                                pvt[:],
                                vsb[t - 1][:, (HD + 1) * h : (HD + 1) * (h + 1)],
                                prs[t - 1][:, 512 * half : 512 * (half + 1)],
                                start=(t - 1 == 0), stop=False,
                            )
                # filler work to absorb the ACT-vs-PE rate gap
                popped += fill_budget
                while fillers and popped >= 1.0:
                    fillers.pop(0)()
                    popped -= 1.0
                # last PV
                for half, pvt in ((0, pv0), (1, pv1)):
                    nc.tensor.matmul(
                        pvt[:],
                        vsb[kt - 1][:, (HD + 1) * h : (HD + 1) * (h + 1)],
                        prs[kt - 1][:, 512 * half : 512 * (half + 1)],
                        start=(kt == 1), stop=True,
                    )
                # extract attention rows + denominator row.  For the final
                # chunk's last heads, use the (idle) ACT engine so the DVE
                # backlog doesn't delay the tail's reciprocal.
                use_act = last and h >= H - 3
                for half, pvt in ((0, pv0), (1, pv1)):
                    att_dst = att_tiles[mt][off : off + HD,
                                            512 * half : 512 * (half + 1)]
                    if use_act:
                        nc.scalar.activation(
                            att_dst, pvt[0:HD, :],
                            mybir.ActivationFunctionType.Copy,
                        )
                    else:
                        nc.vector.tensor_copy(att_dst, pvt[0:HD, :])
                    # den row: compute engines must write at a 32-aligned
                    # partition base, so bounce through a base-0 staging tile,
                    # then DMA (no partition alignment restriction) into row h.
                    stage = probs_pool.tile(
                        [1, 512], F32, name="dstage", tag="dstage"
                    )
                    if use_act:
                        nc.scalar.activation(
                            stage[:], pvt[HD : HD + 1, :],
                            mybir.ActivationFunctionType.Copy,
                        )
                    else:
                        nc.vector.tensor_copy(stage[:], pvt[HD : HD + 1, :])
                    nc.sync.dma_start(
                        den[h : h + 1, 512 * half : 512 * (half + 1)], stage[:]
                    )
            # flush remaining fillers
            while fillers:
                fillers.pop(0)()

        # ---------------- main schedule ----------------
        x0 = emit_x_load(0)
        for th in q_build_thunks(0, x0, wide=True):
            th()

        x1 = emit_x_load(1)
        attention(0, q_build_thunks(1, x1))
        attention(1, normalize_thunks(0) + oproj_thunks(0), last=True)
        for th in normalize_thunks(1, tail=True):
            th()
        for th in oproj_thunks(1, tail=True):
            th()


def _prep_inputs(x, context, context_mask, Wq, Wk, Wv, Wp, bp, q_norm_w, k_norm_w):
    scale = HD ** -0.5
    # selwq[h, 128m + 64j + e] = qw[e]*kw[e]*scale for h = 2m+j (the q-side
    # normalize broadcast selector, carrying both norm weights + softmax scale)
    ww = (q_norm_w.astype(np.float64) * k_norm_w.astype(np.float64) * scale)
    selwq = np.zeros((H, D), np.float64)
    selr = np.zeros((H, D), np.float64)
    for m in range(DT):
        for j in range(2):
            h = 2 * m + j
            c0 = P * m + 64 * j
            selwq[h, c0 : c0 + 64] = ww
            selr[h, c0 : c0 + 64] = 1.0
    shared = {
        "wqT": np.ascontiguousarray(Wq.T).astype(BFNP),
        "wkT": np.ascontiguousarray(Wk.T).astype(BFNP),
        "wvT": np.ascontiguousarray(Wv.T).astype(BFNP),
        "wpT": np.ascontiguousarray(Wp.T).astype(BFNP),
        "bp": bp.reshape(1, D).astype(BFNP),
        "selwq": selwq.astype(BFNP),
        "selr": selr.astype(BFNP),
    }
    # context compaction: keep only unmasked positions (masked ones cannot
    # influence the output), pad to a common multiple of 128.
    idxs = []
    for b in range(B):
        m = context_mask[b].astype(bool).copy()
        if not m.any():
            m[0] = True
        idxs.append(np.nonzero(m)[0])
    lkv_e = max(128, -(-max(len(ix) for ix in idxs) // P) * P)

    in_maps = []
    for b in range(B):
        ix = idxs[b]
        nv = len(ix)
        ctx_c = np.zeros((lkv_e, D), np.float32)
        ctx_c[:nv] = context[b][ix]
        bias = np.full(lkv_e, NEG, np.float32)
        bias[:nv] = 0.0
        in_maps.append(
            dict(
                shared,
                xT=np.ascontiguousarray(x[b].T).astype(BFNP),
                ctxT=np.ascontiguousarray(ctx_c.T).astype(BFNP),
                mask=np.ascontiguousarray(bias.reshape(lkv_e // P, P).T),
            )
        )
    return in_maps, lkv_e


def kernel(x, context, context_mask, Wq, Wk, Wv, Wp, bp, q_norm_w, k_norm_w):
    global LAST_RESULTS
    x = np.asarray(x, dtype=np.float32)
    context = np.asarray(context, dtype=np.float32)
    context_mask = np.asarray(context_mask)
    in_maps, lkv_e = _prep_inputs(
        x, context, context_mask,
        np.asarray(Wq, np.float32), np.asarray(Wk, np.float32),
        np.asarray(Wv, np.float32), np.asarray(Wp, np.float32),
        np.asarray(bp, np.float32), np.asarray(q_norm_w, np.float32),
        np.asarray(k_norm_w, np.float32),
    )
    if lkv_e not in _CACHE:
        _CACHE[lkv_e] = _build(lkv_e)
    nc = _CACHE[lkv_e]
    res = bass_utils.run_bass_kernel_spmd(nc, in_maps, core_ids=list(range(B)))
    LAST_RESULTS = res
    return np.stack([res.results[b]["out"] for b in range(B)], axis=0)


# revision 33
# speedup vs baseline: 1.0349x; 1.0349x over previous
"""Trainium2 Bass kernel for nn_CrossAttention.

Sharding: data-parallel over batch (B=8 -> 8 cores, one batch element per
core). No collectives. Host pre-transposes activations/weights into
contraction-on-partition layouts and casts to bf16.

Performance structure:

  * context compaction: masked-out kv positions (typically ~50%) are
    gathered away on the host; the kernel is compiled at runtime for the
    padded valid length (multiple of 128), cutting scores/PV/K/V matmul
    rows and softmax-exp work roughly in half with bit-identical math for
    the surviving positions (padding rows are killed by the exp bias);
  * attention inner loop is software-pipelined: scores(h,t+1) issue ahead
    of PV(h,t), with exp(h,t) on ACT in between (2-deep PSUM rotation);
  * projection matmul groups (Q of the next chunk, out-proj of the
    previous chunk, normalize broadcasts) are interleaved as PE filler so
    the tensor engine never idles;
  * no DRAM bounces: all partition broadcasts are PE selector matmuls;
  * the k-side RMS factor rides the exp's per-partition scale operand, so
    khat needs no normalize multiply at all;
  * softmax denominators are staged out and inverted per chunk on DVE;
    the projection bias is a DVE add (no PE bias matmuls).
"""

import sys

for _p in ("/opt/trn_rl_repo",):
    if _p not in sys.path:
        sys.path.insert(0, _p)

import numpy as np
import ml_dtypes

import concourse.bass as bass
import concourse.mybir as mybir
import concourse.tile as tile
from concourse import bacc
from concourse import bass_utils

BF16 = mybir.dt.bfloat16
F32 = mybir.dt.float32
BFNP = ml_dtypes.bfloat16

B, LQ, LKV, D, H = 8, 2048, 1024, 1024, 16
HD = D // H          # 64
P = 128              # partitions
DT = D // P          # 8 d-tiles
CH = 1024            # lq chunk
NCH = LQ // CH       # 2
EPS = 1e-6
NEG = -1.0e30

_CACHE = {}
LAST_RESULTS = None


def _patch_act_tables():
    """Restrict usable ACT function sets to natural_log_exp_and_others (it
    contains both Exp and Ln) so the table-load pass never alternates between
    exp_and_others / natural_log — each switch costs ~2.7us on ScalarE."""
    import concourse.hw_specs as hw_specs
    import concourse.bass_interp as bass_interp

    if getattr(_patch_act_tables, "_done", False):
        return
    orig = hw_specs.get_activation_tables

    def patched(module_arch):
        t = orig(module_arch)
        keep = "natural_log_exp_and_others"
        if keep in t:
            t = {k: (v if k == keep else set()) for k, v in t.items()}
        return t

    hw_specs.get_activation_tables = patched
    bacc.get_activation_tables = patched
    bass_interp.get_activation_tables = patched
    _patch_act_tables._done = True


def _build(lkv_e):
    _patch_act_tables()
    nc = bacc.Bacc("TRN2", target_bir_lowering=False, debug=False)

    kt = lkv_e // P
    ctxT_d = nc.dram_tensor("ctxT", (D, lkv_e), BF16, kind="ExternalInput").ap()
    xT_d = nc.dram_tensor("xT", (D, LQ), BF16, kind="ExternalInput").ap()
    wqT_d = nc.dram_tensor("wqT", (D, D), BF16, kind="ExternalInput").ap()
    wkT_d = nc.dram_tensor("wkT", (D, D), BF16, kind="ExternalInput").ap()
    wvT_d = nc.dram_tensor("wvT", (D, D), BF16, kind="ExternalInput").ap()
    wpT_d = nc.dram_tensor("wpT", (D, D), BF16, kind="ExternalInput").ap()
    bp_d = nc.dram_tensor("bp", (1, D), BF16, kind="ExternalInput").ap()
    mask_d = nc.dram_tensor("mask", (P, kt), F32, kind="ExternalInput").ap()
    selwq_d = nc.dram_tensor("selwq", (H, D), BF16, kind="ExternalInput").ap()
    selr_d = nc.dram_tensor("selr", (H, D), BF16, kind="ExternalInput").ap()
    out_d = nc.dram_tensor("out", (LQ, D), F32, kind="ExternalOutput").ap()

    with tile.TileContext(nc) as tc:
        _kernel_body(
            nc, tc, lkv_e, xT_d, ctxT_d, wqT_d, wkT_d, wvT_d, wpT_d, bp_d,
            mask_d, selwq_d, selr_d, out_d,
        )
    nc.compile()
    return nc


def _kernel_body(
    nc, tc, lkv_e, xT_d, ctxT_d, wqT_d, wkT_d, wvT_d, wpT_d, bp_d, mask_d,
    selwq_d, selr_d, out_d,
):
    import contextlib

    kt = lkv_e // P
    # K-projection free-dim slices of the compacted kv length
    kslices = []
    o = 0
    while o < lkv_e:
        w = min(512, lkv_e - o)
        kslices.append((o, w))
        o += w

    ctx = contextlib.ExitStack()
    with ctx:
        const = ctx.enter_context(tc.tile_pool(name="const", bufs=1))
        wpool = ctx.enter_context(tc.tile_pool(name="wpool", bufs=1))
        io = ctx.enter_context(tc.tile_pool(name="io", bufs=2))
        kv = ctx.enter_context(tc.tile_pool(name="kv", bufs=1))
        sqp = ctx.enter_context(tc.tile_pool(name="sqp", bufs=1))
        qp = ctx.enter_context(tc.tile_pool(name="qp", bufs=2))
        va = ctx.enter_context(tc.tile_pool(name="va", bufs=2))
        probs_pool = ctx.enter_context(tc.tile_pool(name="probs", bufs=3))
        small = ctx.enter_context(tc.tile_pool(name="small", bufs=1))
        den_pool = ctx.enter_context(tc.tile_pool(name="den", bufs=1))
        out_pool = ctx.enter_context(tc.tile_pool(name="outp", bufs=2))
        sc_ps = ctx.enter_context(tc.tile_pool(name="sc_ps", bufs=2, space="PSUM"))
        pv_ps = ctx.enter_context(tc.tile_pool(name="pv_ps", bufs=2, space="PSUM"))
        aux_ps = ctx.enter_context(tc.tile_pool(name="aux_ps", bufs=2, space="PSUM"))

        # ---- loads needed first: wk + compacted ctx (K-proj inputs) ----
        wk_sb = []
        for k in range(DT):
            t = wpool.tile([P, D], BF16, name=f"wk{k}", tag=f"wkp{k}")
            nc.sync.dma_start(t[:], wkT_d[P * k : P * (k + 1), :])
            wk_sb.append(t)
        ctx_sb = []
        for k in range(DT):
            t = io.tile([P, lkv_e], BF16, name=f"ctx{k}", tag=f"io{k}")
            nc.sync.dma_start(t[:], ctxT_d[P * k : P * (k + 1), :])
            ctx_sb.append(t)

        # ---- small constants ----
        mask_sb = const.tile([P, kt], F32, name="mask_sb")
        nc.sync.dma_start(mask_sb[:], mask_d[:])
        selwq_sb = const.tile([H, D], BF16, name="selwq_sb")
        nc.sync.dma_start(selwq_sb[:], selwq_d[:])
        selr = const.tile([H, D], BF16, name="selr")
        nc.sync.dma_start(selr[:], selr_d[:])
        bias_sb = const.tile([P, D], BF16, name="bias_sb")
        nc.sync.dma_start(bias_sb[:], bp_d[0:1, :].broadcast_to((P, D)))
        eps128 = const.tile([P, 1], F32, name="eps128")
        nc.vector.memset(eps128[:], EPS)
        zero128 = const.tile([P, 1], F32, name="zero128")
        nc.vector.memset(zero128[:], 0.0)
        eps16 = const.tile([H, 1], F32, name="eps16")
        nc.vector.memset(eps16[:], EPS)
        zero16 = const.tile([H, 1], F32, name="zero16")
        nc.vector.memset(zero16[:], 0.0)
        # sel16[m]: [128, 16] with col 2m set on partitions 0-63, col 2m+1 on
        # 64-127.  Used as rhs for k-stats and lhsT for q-stats.
        sel16 = []
        for m in range(DT):
            s = const.tile([P, H], BF16, name=f"sel{m}")
            nc.vector.memset(s[:], 0.0)
            nc.vector.memset(s[0:64, 2 * m : 2 * m + 1], 1.0)
            nc.vector.memset(s[64:128, 2 * m + 1 : 2 * m + 2], 1.0)
            sel16.append(s)

        # remaining weights: wv shares slots with the attn accumulators
        wv_sb = []
        for k in range(DT):
            t = va.tile([P, D], BF16, name=f"wv{k}", tag=f"va{k}")
            nc.sync.dma_start(t[:], wvT_d[P * k : P * (k + 1), :])
            wv_sb.append(t)
        wq_sb = []
        for k in range(DT):
            t = wpool.tile([P, D], BF16, name=f"wq{k}", tag=f"wq{k}")
            nc.sync.dma_start(t[:], wqT_d[P * k : P * (k + 1), :])
            wq_sb.append(t)

        khat = [kv.tile([P, lkv_e], BF16, name=f"khat{m}") for m in range(DT)]
        vsb = [kv.tile([P, H * (HD + 1)], BF16, name=f"vsb{m}") for m in range(kt)]

        def make_ps_alloc(wide):
            """PSUM allocator for projection groups.  While attention is NOT
            running (pre-attention stages, final tail), cycle through the idle
            sc/pv slots too, for a 6-deep rotation so the PE runs well ahead
            of the DVE copies.  During attention only the aux slots are safe."""
            import itertools

            if not wide:
                seq = itertools.cycle([(aux_ps, "mm")])
            else:
                seq = itertools.cycle(
                    [(aux_ps, "mm"), (sc_ps, "sc"), (aux_ps, "mm"), (pv_ps, "pv")]
                )

            def alloc(name):
                pool, tag = next(seq)
                return pool.tile([P, 512], F32, name=name, tag=tag)

            return alloc

        # ---------------- K projection ----------------
        pre_alloc = make_ps_alloc(True)
        sq_k = []
        for m in range(DT):
            for o, w in kslices:
                ps = pre_alloc("ps_k")
                for k in range(DT):
                    nc.tensor.matmul(
                        ps[:, 0:w],
                        wk_sb[k][:, P * m : P * (m + 1)],
                        ctx_sb[k][:, o : o + w],
                        start=(k == 0), stop=(k == DT - 1),
                    )
                nc.vector.tensor_copy(khat[m][:, o : o + w], ps[:, 0:w])
            sq = sqp.tile([P, lkv_e], BF16, name=f"sq{m}", tag=f"sq{m}")
            nc.vector.tensor_tensor(
                sq[:], khat[m][:], khat[m][:], mybir.AluOpType.mult
            )
            sq_k.append(sq)

        # wp loads into wk's slots once K-proj has consumed wk.
        wp_sb = []
        for k in range(DT):
            t = wpool.tile([P, D], BF16, name=f"wp{k}", tag=f"wkp{k}")
            nc.sync.dma_start(t[:], wpT_d[P * k : P * (k + 1), :])
            wp_sb.append(t)

        # ---------------- V projection ----------------
        for m in range(kt):
            for n in range(2):
                ps = pre_alloc("ps_v")
                for k in range(DT):
                    nc.tensor.matmul(
                        ps[:],
                        ctx_sb[k][:, P * m : P * (m + 1)],
                        wv_sb[k][:, 512 * n : 512 * (n + 1)],
                        start=(k == 0), stop=(k == DT - 1),
                    )
                v3 = vsb[m][:].rearrange("p (h e) -> p h e", e=HD + 1)
                nc.vector.tensor_copy(
                    v3[:, 8 * n : 8 * (n + 1), 0:HD],
                    ps[:].rearrange("p (h e) -> p h e", e=HD),
                )
            v3 = vsb[m][:].rearrange("p (h e) -> p h e", e=HD + 1)
            nc.vector.memset(v3[:, :, HD : HD + 1], 1.0)

        # ---------------- k-side RMS factors (transposed stats) ----------
        # rkps[kpos, 16t + h] = sum_d k[d, kpos]^2 for head h, lkv tile t.
        rkps = pv_ps.tile([P, H * kt], F32, name="rkps", tag="pv")
        for t in range(kt):
            for m in range(DT):
                nc.tensor.matmul(
                    rkps[:, H * t : H * (t + 1)],
                    sq_k[m][:, P * t : P * (t + 1)],
                    sel16[m][:],
                    start=(m == 0), stop=(m == DT - 1),
                )
        rkln = small.tile([P, H * kt], F32, name="rkln")
        nc.scalar.activation(
            rkln[:], rkps[:], mybir.ActivationFunctionType.Ln,
            bias=eps128[:], scale=1.0 / HD,
        )
        rk_sb = small.tile([P, H * kt], F32, name="rk_sb")
        nc.scalar.activation(
            rk_sb[:], rkln[:], mybir.ActivationFunctionType.Exp,
            bias=zero128[:], scale=-0.5,
        )

        # ---------------- per-chunk helpers ----------------
        qhat = {}      # chunk -> list of 8 [128, CH] bf16 tiles (normalized q)
        attn = {}      # chunk -> list of 8 [128, CH] bf16 tiles
        denall = {}    # chunk -> [16, CH] f32 denominators

        def emit_x_load(c):
            tiles = []
            for k in range(DT):
                t = io.tile([P, CH], BF16, name=f"x{c}_{k}", tag=f"io{k}")
                nc.sync.dma_start(
                    t[:], xT_d[P * k : P * (k + 1), CH * c : CH * (c + 1)]
                )
                tiles.append(t)
            return tiles

        def q_build_thunks(c, x_tiles, wide=False):
            """Thunk list: Q projection + RMS stats + normalize for chunk c."""
            ps_alloc = make_ps_alloc(wide)
            q_tiles = [
                qp.tile([P, CH], BF16, name=f"q{c}_{m}", tag=f"q{m}")
                for m in range(DT)
            ]
            qhat[c] = q_tiles
            sq_q = []
            thunks = []

            def proj_group(m, n):
                def f():
                    ps = ps_alloc("ps_q")
                    for k in range(DT):
                        nc.tensor.matmul(
                            ps[:],
                            wq_sb[k][:, P * m : P * (m + 1)],
                            x_tiles[k][:, 512 * n : 512 * (n + 1)],
                            start=(k == 0), stop=(k == DT - 1),
                        )
                    nc.vector.tensor_copy(
                        q_tiles[m][:, 512 * n : 512 * (n + 1)], ps[:]
                    )
                    if n == 1:
                        sq = sqp.tile([P, CH], BF16, name=f"sqq{c}_{m}", tag=f"sq{m}")
                        nc.vector.tensor_tensor(
                            sq[:], q_tiles[m][:], q_tiles[m][:],
                            mybir.AluOpType.mult,
                        )
                        sq_q.append(sq)
                return f

            for m in range(DT):
                for n in range(2):
                    thunks.append(proj_group(m, n))

            ln_sb = small.tile([H, CH], F32, name=f"lnq{c}", tag="lnq")
            rs_q = small.tile([H, CH], BF16, name=f"rsq{c}", tag="rsq")

            def stats():
                for n in range(2):
                    qs = ps_alloc("qs")[0:H, :]
                    for m in range(DT):
                        nc.tensor.matmul(
                            qs[:],
                            sel16[m][:],
                            sq_q[m][:, 512 * n : 512 * (n + 1)],
                            start=(m == 0), stop=(m == DT - 1),
                        )
                    nc.scalar.activation(
                        ln_sb[:, 512 * n : 512 * (n + 1)], qs[:],
                        mybir.ActivationFunctionType.Ln,
                        bias=eps16[:], scale=1.0 / HD,
                    )
                nc.scalar.activation(
                    rs_q[:], ln_sb[:], mybir.ActivationFunctionType.Exp,
                    bias=zero16[:], scale=-0.5,
                )

            thunks.append(stats)

            def bcast_group(m):
                def f():
                    for n in range(2):
                        rb = ps_alloc("rbq")
                        nc.tensor.matmul(
                            rb[:],
                            selwq_sb[:, P * m : P * (m + 1)],
                            rs_q[:, 512 * n : 512 * (n + 1)],
                            start=True, stop=True,
                        )
                        nc.vector.tensor_tensor(
                            q_tiles[m][:, 512 * n : 512 * (n + 1)],
                            q_tiles[m][:, 512 * n : 512 * (n + 1)],
                            rb[:],
                            mybir.AluOpType.mult,
                        )
                return f

            for m in range(DT):
                thunks.append(bcast_group(m))
            return thunks

        def normalize_thunks(c, tail=False):
            """Reciprocal of denominators + broadcast + apply to attn[c]."""
            thunks = []
            ps_alloc = make_ps_alloc(tail)
            recf = small.tile([H, CH], F32, name=f"recf{c}", tag="recf")
            rec16 = small.tile([H, CH], BF16, name=f"rec{c}", tag="rec16")

            def recip():
                nc.vector.reciprocal_approx_fast(recf[:], denall[c][:])
                nc.vector.tensor_copy(rec16[:], recf[:])

            thunks.append(recip)

            def rbr_group(m):
                def f():
                    for n in range(2):
                        rb = ps_alloc("rbr")
                        nc.tensor.matmul(
                            rb[:],
                            selr[:, P * m : P * (m + 1)],
                            rec16[:, 512 * n : 512 * (n + 1)],
                            start=True, stop=True,
                        )
                        nc.vector.tensor_tensor(
                            attn[c][m][:, 512 * n : 512 * (n + 1)],
                            attn[c][m][:, 512 * n : 512 * (n + 1)],
                            rb[:],
                            mybir.AluOpType.mult,
                        )
                return f

            for m in range(DT):
                thunks.append(rbr_group(m))
            return thunks

        def oproj_thunks(c, tail=False):
            thunks = []
            ps_alloc = make_ps_alloc(tail)

            def group(mq, n):
                def f():
                    ps = ps_alloc("ps_o")
                    for k in range(DT):
                        nc.tensor.matmul(
                            ps[:],
                            attn[c][k][:, P * mq : P * (mq + 1)],
                            wp_sb[k][:, 512 * n : 512 * (n + 1)],
                            start=(k == 0), stop=(k == DT - 1),
                        )
                    o_sb = out_pool.tile([P, 512], F32, name="o_sb", tag="o")
                    nc.vector.tensor_tensor(
                        o_sb[:], ps[:], bias_sb[:, 512 * n : 512 * (n + 1)],
                        mybir.AluOpType.add,
                    )
                    nc.sync.dma_start(
                        out_d[CH * c + P * mq : CH * c + P * (mq + 1),
                              512 * n : 512 * (n + 1)],
                        o_sb[:],
                    )
                return f

            for mq in range(CH // P):
                for n in range(2):
                    thunks.append(group(mq, n))
            return thunks

        def attention(c, fillers, last=False):
            """Attention for chunk c; pops filler thunks to keep PE fed."""
            att_tiles = [
                va.tile([P, CH], BF16, name=f"at{c}_{m}", tag=f"va{m}")
                for m in range(DT)
            ]
            attn[c] = att_tiles
            den = den_pool.tile([H, CH], F32, name=f"den{c}", tag="den")
            denall[c] = den

            # hold a few thunks back for after the last head, so the PE has
            # work while the final denominator -> reciprocal chain drains.
            fill_budget = len(fillers) / 18.0 / 2.0
            popped = 0.0
            for h in range(H):
                mt, off = h // 2, 64 * (h % 2)
                pv0 = pv_ps.tile([HD + 1, 512], F32, name="pv0", tag="pv")
                pv1 = pv_ps.tile([HD + 1, 512], F32, name="pv1", tag="pv")
                prs = []
                for t in range(kt):
                    if t == kt // 2:
                        popped += fill_budget
                        while fillers and popped >= 1.0:
                            fillers.pop(0)()
                            popped -= 1.0
                    sc = sc_ps.tile([P, CH], F32, name="sc", tag="sc")
                    for n in range(2):
                        nc.tensor.matmul(
                            sc[:, 512 * n : 512 * (n + 1)],
                            khat[mt][off : off + HD, P * t : P * (t + 1)],
                            qhat[c][mt][off : off + HD, 512 * n : 512 * (n + 1)],
                            start=True, stop=True,
                        )
                    pr = probs_pool.tile([P, CH], BF16, name="pr", tag="pr")
                    nc.scalar.activation(
                        pr[:], sc[:], mybir.ActivationFunctionType.Exp,
                        bias=mask_sb[:, t : t + 1],
                        scale=rk_sb[:, H * t + h : H * t + h + 1],
                    )
                    prs.append(pr)
                    if t >= 1:
                        for half, pvt in ((0, pv0), (1, pv1)):
                            nc.tensor.matmul(
                                pvt[:],
                                vsb[t - 1][:, (HD + 1) * h : (HD + 1) * (h + 1)],
                                prs[t - 1][:, 512 * half : 512 * (half + 1)],
                                start=(t - 1 == 0), stop=False,
                            )
                # filler work to absorb the ACT-vs-PE rate gap
                popped += fill_budget
                while fillers and popped >= 1.0:
                    fillers.pop(0)()
                    popped -= 1.0
                # last PV
                for half, pvt in ((0, pv0), (1, pv1)):
                    nc.tensor.matmul(
                        pvt[:],
                        vsb[kt - 1][:, (HD + 1) * h : (HD + 1) * (h + 1)],
                        prs[kt - 1][:, 512 * half : 512 * (half + 1)],
                        start=(kt == 1), stop=True,
                    )
                # extract attention rows + denominator row.  For the final
                # chunk's last heads, use the (idle) ACT engine so the DVE
                # backlog doesn't delay the tail's reciprocal.
                use_act = last and h >= H - 3
                for half, pvt in ((0, pv0), (1, pv1)):
                    att_dst = att_tiles[mt][off : off + HD,
                                            512 * half : 512 * (half + 1)]
                    if use_act:
                        nc.scalar.activation(
                            att_dst, pvt[0:HD, :],
                            mybir.ActivationFunctionType.Copy,
                        )
                    else:
                        nc.vector.tensor_copy(att_dst, pvt[0:HD, :])
                    # den row: compute engines must write at a 32-aligned
                    # partition base, so bounce through a base-0 staging tile,
                    # then DMA (no partition alignment restriction) into row h.
                    stage = probs_pool.tile(
                        [1, 512], F32, name="dstage", tag="dstage"
                    )
                    if use_act:
                        nc.scalar.activation(
                            stage[:], pvt[HD : HD + 1, :],
                            mybir.ActivationFunctionType.Copy,
                        )
                    else:
                        nc.vector.tensor_copy(stage[:], pvt[HD : HD + 1, :])
                    nc.sync.dma_start(
                        den[h : h + 1, 512 * half : 512 * (half + 1)], stage[:]
                    )
            # flush remaining fillers
            while fillers:
                fillers.pop(0)()

        # ---------------- main schedule ----------------
        x0 = emit_x_load(0)
        for th in q_build_thunks(0, x0, wide=True):
            th()

        x1 = emit_x_load(1)
        attention(0, q_build_thunks(1, x1))
        attention(1, normalize_thunks(0) + oproj_thunks(0), last=True)
        for th in normalize_thunks(1, tail=True):
            th()
        for th in oproj_thunks(1, tail=True):
            th()


def _prep_inputs(x, context, context_mask, Wq, Wk, Wv, Wp, bp, q_norm_w, k_norm_w):
    scale = HD ** -0.5
    # selwq[h, 128m + 64j + e] = qw[e]*kw[e]*scale for h = 2m+j (the q-side
    # normalize broadcast selector, carrying both norm weights + softmax scale)
    ww = (q_norm_w.astype(np.float64) * k_norm_w.astype(np.float64) * scale)
    selwq = np.zeros((H, D), np.float64)
    selr = np.zeros((H, D), np.float64)
    for m in range(DT):
        for j in range(2):
            h = 2 * m + j
            c0 = P * m + 64 * j
            selwq[h, c0 : c0 + 64] = ww
            selr[h, c0 : c0 + 64] = 1.0
    shared = {
        "wqT": np.ascontiguousarray(Wq.T).astype(BFNP),
        "wkT": np.ascontiguousarray(Wk.T).astype(BFNP),
        "wvT": np.ascontiguousarray(Wv.T).astype(BFNP),
        "wpT": np.ascontiguousarray(Wp.T).astype(BFNP),
        "bp": bp.reshape(1, D).astype(BFNP),
        "selwq": selwq.astype(BFNP),
        "selr": selr.astype(BFNP),
    }
    # context compaction: keep only unmasked positions (masked ones cannot
    # influence the output), pad to a common multiple of 128.
    idxs = []
    for b in range(B):
        m = context_mask[b].astype(bool).copy()
        if not m.any():
            m[0] = True
        idxs.append(np.nonzero(m)[0])
    lkv_e = max(128, -(-max(len(ix) for ix in idxs) // P) * P)

    in_maps = []
    for b in range(B):
        ix = idxs[b]
        nv = len(ix)
        ctx_c = np.zeros((lkv_e, D), np.float32)
        ctx_c[:nv] = context[b][ix]
        bias = np.full(lkv_e, NEG, np.float32)
        bias[:nv] = 0.0
        in_maps.append(
            dict(
                shared,
                xT=np.ascontiguousarray(x[b].T).astype(BFNP),
                ctxT=np.ascontiguousarray(ctx_c.T).astype(BFNP),
                mask=np.ascontiguousarray(bias.reshape(lkv_e // P, P).T),
            )
        )
    return in_maps, lkv_e


def kernel(x, context, context_mask, Wq, Wk, Wv, Wp, bp, q_norm_w, k_norm_w):
    global LAST_RESULTS
    x = np.asarray(x, dtype=np.float32)
    context = np.asarray(context, dtype=np.float32)
    context_mask = np.asarray(context_mask)
    in_maps, lkv_e = _prep_inputs(
        x, context, context_mask,
        np.asarray(Wq, np.float32), np.asarray(Wk, np.float32),
        np.asarray(Wv, np.float32), np.asarray(Wp, np.float32),
        np.asarray(bp, np.float32), np.asarray(q_norm_w, np.float32),
        np.asarray(k_norm_w, np.float32),
    )
    if lkv_e not in _CACHE:
        _CACHE[lkv_e] = _build(lkv_e)
    nc = _CACHE[lkv_e]
    res = bass_utils.run_bass_kernel_spmd(nc, in_maps, core_ids=list(range(B)))
    LAST_RESULTS = res
    return np.stack([res.results[b]["out"] for b in range(B)], axis=0)
